# revision 6
# baseline (speedup 1.0000x reference)
"""Trainium2 Bass kernel for nn_MihGNNEmbeddingTest1 (gnn_message_passing).

Strategy (8 cores):
  - Row-shard A_s: core c owns rows [c*1024, (c+1)*1024). Host passes
    A_s[rows].T as contiguous fp16 (values of A_s are exactly fp16).
  - Each core computes P_c^T = emb^T @ A_s[rows]^T on PE (emb stationary,
    fp16, f32 PSUM accumulate), PE-transposes to row-major P_c, and builds
    local table rows T_c[m] = [emb[m] | P_c[m] | pad] (128 fp16 = 256B).
  - AllGather T_c -> full table T [8192, 128] fp16 on every core.
  - Per-edge gather via transpose-mode dma_gather (one instr per endpoint,
    4096 idxs): gathered features land feature-on-partition: rows 0:32 =
    emb^T, 32:64 = P^T of the gathered nodes.
  - label select folded into MLP: rhs = gathered[0:64] * mask, lhsT =
    [W_T ; W_T] so sum splits into (1-l)*W@emb + l*W@P.
  - 4 Linear layers stacked on 128 output partitions, ReLU+bias on ACT,
    layer-sum and src-dst subtraction via a +/- stacked-identity matmul,
    squared-distance partition-reduce via one-hot matmul accumulating all
    8 chunks into one [8, 512] PSUM tile, exp on ACT, loss reduced on-chip.
Outputs: per-core predicts [8,512] (edge shard) and partial loss [1,1];
host concatenates predicts and sums losses.
"""

import sys

import numpy as np

if "/opt/trn_rl_repo" not in sys.path:
    sys.path.insert(0, "/opt/trn_rl_repo")

N, D, B, L = 8192, 32, 32768, 4
NCORES = 8
M_LOC = N // NCORES          # 1024 A_s rows per core
B_LOC = B // NCORES          # 4096 edges per core
KT = N // 128                # 64 contraction tiles
G = 4                        # k-tiles per DMA slab
NSLAB = KT // G              # 16
NCHUNK = B_LOC // 512        # 8 edge chunks of 512
_prog_cache = {}


def _build_program():
    import concourse.bacc as bacc
    import concourse.mybir as mybir
    import concourse.tile as tile

    fp16 = mybir.dt.float16
    f32 = mybir.dt.float32
    i16 = mybir.dt.int16
    AF = mybir.ActivationFunctionType

    nc = bacc.Bacc("TRN2", target_bir_lowering=False, debug=False,
                   num_devices=NCORES)

    asT = nc.dram_tensor("asT", [N, M_LOC], fp16, kind="ExternalInput")
    embT_d = nc.dram_tensor("embT", [N, D], fp16, kind="ExternalInput")
    embloc_d = nc.dram_tensor("embloc", [M_LOC, D], fp16, kind="ExternalInput")
    idxs_d = nc.dram_tensor("idxs", [128, 2, B_LOC // 16], i16,
                            kind="ExternalInput")
    mask_d = nc.dram_tensor("mask", [64, B_LOC], fp16, kind="ExternalInput")
    labels_d = nc.dram_tensor("labels_t", [NCHUNK, 512], f32,
                              kind="ExternalInput")
    w2t_d = nc.dram_tensor("w2t", [64, 128], f32, kind="ExternalInput")
    bstack_d = nc.dram_tensor("bstack", [128, 1], f32, kind="ExternalInput")
    spm_d = nc.dram_tensor("spm", [128, 64], f32, kind="ExternalInput")
    oh_d = nc.dram_tensor("oh", [32, NCHUNK * 8], f32, kind="ExternalInput")
    ones8_d = nc.dram_tensor("ones8", [8, 1], f32, kind="ExternalInput")
    ident_d = nc.dram_tensor("ident", [32, 32], fp16, kind="ExternalInput")

    predicts_d = nc.dram_tensor("predicts", [NCHUNK, 512], f32,
                                kind="ExternalOutput")
    loss_d = nc.dram_tensor("loss", [1, 1], f32, kind="ExternalOutput")

    tloc_d = nc.dram_tensor("tloc", [M_LOC, 128], fp16)
    tfull_d = nc.dram_tensor("tfull", [N, 128], fp16, addr_space="Shared")

    with tile.TileContext(nc) as tc:
        with tc.tile_pool(name="const", bufs=1) as constp:
            embT_sb = constp.tile([128, KT, D], fp16)
            nc.sync.dma_start(
                out=embT_sb[:],
                in_=embT_d.rearrange("(kt p) d -> p kt d", p=128))
            embloc_sb = constp.tile([128, 8, D], fp16)
            nc.sync.dma_start(
                out=embloc_sb[:],
                in_=embloc_d.rearrange("(t p) d -> p t d", p=128))
            ident_sb = constp.tile([32, 32], fp16)
            nc.sync.dma_start(out=ident_sb[:], in_=ident_d[:])

            # ---------------- phase 1: P^T = emb^T @ A_s^T ----------------
            with tc.tile_pool(name="asp", bufs=3) as asp, \
                 tc.tile_pool(name="pt_ps", bufs=2, space="PSUM") as pspt, \
                 tc.tile_pool(name="tr_ps", bufs=2, space="PSUM") as pstr, \
                 tc.tile_pool(name="ph1", bufs=1) as ph1:
                pts = [pspt.tile([32, 512], f32, tag=f"pt{mc}", name=f"pt{mc}")
                       for mc in range(2)]
                asr = asT.rearrange("(s g p) m -> s p g m", p=128, g=G)
                for s in range(NSLAB):
                    aslab = asp.tile([128, G, M_LOC], fp16, tag="aslab")
                    nc.sync.dma_start(out=aslab[:], in_=asr[s])
                    for g in range(G):
                        kt = s * G + g
                        for mc in range(2):
                            nc.tensor.matmul(
                                out=pts[mc][:],
                                lhsT=embT_sb[:, kt, :],
                                rhs=aslab[:, g, mc * 512:(mc + 1) * 512],
                                start=(kt == 0), stop=(kt == KT - 1))
                ptsb = ph1.tile([32, M_LOC], fp16)
                for mc in range(2):
                    nc.scalar.copy(out=ptsb[:, mc * 512:(mc + 1) * 512],
                                   in_=pts[mc][:])
                # assemble local table rows [emb | P | pad]
                tc_sb = ph1.tile([128, 8, 128], fp16)
                nc.vector.memset(tc_sb[:], 0.0)
                nc.vector.tensor_copy(out=tc_sb[:, :, 0:D], in_=embloc_sb[:])
                for t in range(8):
                    ptr = pstr.tile([128, 32], fp16, tag="ptr", name="ptr")
                    nc.tensor.transpose(out=ptr[:],
                                        in_=ptsb[:, t * 128:(t + 1) * 128],
                                        identity=ident_sb[:])
                    nc.vector.tensor_copy(out=tc_sb[:, t, 32:64], in_=ptr[:])
                nc.sync.dma_start(
                    out=tloc_d.rearrange("(t p) e -> p t e", p=128),
                    in_=tc_sb[:])

            # ---------------- all-gather the table ----------------
            nc.gpsimd.collective_compute(
                "AllGather", mybir.AluOpType.bypass,
                replica_groups=[list(range(NCORES))],
                ins=[tloc_d[:]], outs=[tfull_d[:]])

            # ---------------- phase 2: gather + MLP + loss ----------------
            with tc.tile_pool(name="sb2", bufs=2) as sb2, \
                 tc.tile_pool(name="c2", bufs=1) as c2, \
                 tc.tile_pool(name="mlp_ps", bufs=3, space="PSUM") as psmlp, \
                 tc.tile_pool(name="g_ps", bufs=2, space="PSUM") as psg, \
                 tc.tile_pool(name="d_ps", bufs=1, space="PSUM") as psd, \
                 tc.tile_pool(name="l_ps", bufs=1, space="PSUM") as psl:
                idx_sb = c2.tile([128, 2, B_LOC // 16], i16)
                nc.sync.dma_start(out=idx_sb[:], in_=idxs_d[:])
                mask_sb = c2.tile([64, B_LOC], fp16)
                nc.sync.dma_start(out=mask_sb[:], in_=mask_d[:])
                labels_sb = c2.tile([NCHUNK, 512], f32)
                nc.sync.dma_start(out=labels_sb[:], in_=labels_d[:])
                w2t_sb = c2.tile([64, 128], f32)
                nc.sync.dma_start(out=w2t_sb[:], in_=w2t_d[:])
                bstack_sb = c2.tile([128, 1], f32)
                nc.sync.dma_start(out=bstack_sb[:], in_=bstack_d[:])
                spm_sb = c2.tile([128, 64], f32)
                nc.sync.dma_start(out=spm_sb[:], in_=spm_d[:])
                oh_sb = c2.tile([32, NCHUNK * 8], f32)
                nc.sync.dma_start(out=oh_sb[:], in_=oh_d[:])
                ones8_sb = c2.tile([8, 1], f32)
                nc.sync.dma_start(out=ones8_sb[:], in_=ones8_d[:])

                gts = []
                for ep in range(2):
                    gt = c2.tile([128, 1, B_LOC], fp16, tag=f"gt{ep}", name=f"gt{ep}")
                    nc.gpsimd.dma_gather(gt[:], tfull_d[:], idx_sb[:, ep, :],
                                         B_LOC, B_LOC, 128, transpose=True,
                                         single_packet=False)
                    gts.append(gt)

                psd_t = psd.tile([8, 512], f32)
                for j in range(NCHUNK):
                    cs = slice(j * 512, (j + 1) * 512)
                    relus = []
                    for ep in range(2):
                        rm = sb2.tile([64, 512], f32, tag=f"rm{ep}", name=f"rm{ep}")
                        nc.vector.tensor_mul(out=rm[:],
                                             in0=gts[ep][0:64, 0, cs],
                                             in1=mask_sb[:, cs])
                        ph = psmlp.tile([128, 512], f32, tag="ph")
                        nc.tensor.matmul(out=ph[:], lhsT=w2t_sb[:], rhs=rm[:],
                                         start=True, stop=True)
                        rl = sb2.tile([128, 512], f32, tag=f"rl{ep}", name=f"rl{ep}")
                        nc.scalar.activation(out=rl[:], in_=ph[:],
                                             func=AF.Relu,
                                             bias=bstack_sb[:, 0:1])
                        relus.append(rl)
                    pg = psg.tile([32, 512], f32, tag="pg")
                    nc.tensor.matmul(out=pg[:], lhsT=spm_sb[:, 0:32],
                                     rhs=relus[0][:], start=True, stop=False)
                    nc.tensor.matmul(out=pg[:], lhsT=spm_sb[:, 32:64],
                                     rhs=relus[1][:], start=False, stop=True)
                    gsq = sb2.tile([32, 512], f32, tag="gsq")
                    nc.scalar.square(out=gsq[:], in_=pg[:])
                    nc.tensor.matmul(out=psd_t[:],
                                     lhsT=oh_sb[:, j * 8:(j + 1) * 8],
                                     rhs=gsq[:],
                                     start=(j == 0), stop=(j == NCHUNK - 1))

                pred_sb = sb2.tile([NCHUNK, 512], f32, tag="pred")
                nc.scalar.activation(out=pred_sb[:], in_=psd_t[:],
                                     func=AF.Exp, scale=-1.0 / D)
                nc.sync.dma_start(out=predicts_d[:], in_=pred_sb[:])

                diff = sb2.tile([NCHUNK, 512], f32, tag="diff")
                nc.vector.tensor_sub(out=diff[:], in0=labels_sb[:],
                                     in1=pred_sb[:])
                sq = sb2.tile([NCHUNK, 512], f32, tag="sq")
                lsum = sb2.tile([NCHUNK, 1], f32, tag="lsum")
                nc.scalar.activation(out=sq[:], in_=diff[:], func=AF.Square,
                                     accum_out=lsum[:])
                pl = psl.tile([1, 1], f32)
                nc.tensor.matmul(out=pl[:], lhsT=ones8_sb[:], rhs=lsum[:],
                                 start=True, stop=True)
                loss_sb = sb2.tile([1, 1], f32, tag="losssb")
                nc.scalar.mul(out=loss_sb[:], in_=pl[:], mul=0.5)
                nc.sync.dma_start(out=loss_d[:], in_=loss_sb[:])

    nc.compile()
    return nc


def _get_program():
    if "nc" not in _prog_cache:
        _prog_cache["nc"] = _build_program()
    return _prog_cache["nc"]


def _host_inputs(edges, labels, A_s, embedding, W, b):
    edges = np.asarray(edges)
    labels = np.asarray(labels, dtype=np.float32)
    A_s = np.asarray(A_s, dtype=np.float32)
    emb = np.asarray(embedding, dtype=np.float32)
    W = np.asarray(W, dtype=np.float32)
    bb = np.asarray(b, dtype=np.float32)
    src = np.asarray(edges[:, 0], dtype=np.int64)
    dst = np.asarray(edges[:, 1], dtype=np.int64)

    emb16 = emb.astype(np.float16)
    wt = np.ascontiguousarray(W.transpose(2, 0, 1).reshape(D, L * D))
    w2t = np.concatenate([wt, wt], axis=0)            # [64, 128]
    bstack = bb.reshape(L * D, 1).astype(np.float32)  # [128, 1]
    spm = np.zeros([128, 64], np.float32)
    eye = np.eye(32, dtype=np.float32)
    for li in range(L):
        spm[li * 32:(li + 1) * 32, 0:32] = eye
        spm[li * 32:(li + 1) * 32, 32:64] = -eye
    oh = np.zeros([32, NCHUNK * 8], np.float32)
    for j in range(NCHUNK):
        oh[:, j * 8 + j] = 1.0
    ones8 = np.ones([8, 1], np.float32)
    ident = np.eye(32, dtype=np.float16)

    iarange = np.arange(B_LOC)

    def idxtile(v):
        a = np.zeros([16, B_LOC // 16], np.int16)
        a[iarange % 16, iarange // 16] = v.astype(np.int16)
        return np.tile(a, (8, 1))

    in_maps = []
    for c in range(NCORES):
        rows = slice(c * M_LOC, (c + 1) * M_LOC)
        es = slice(c * B_LOC, (c + 1) * B_LOC)
        asT_c = np.ascontiguousarray(A_s[rows, :].T.astype(np.float16))
        labc = labels[es]
        mask = np.empty([64, B_LOC], np.float16)
        mask[0:32, :] = (1.0 - labc)[None, :]
        mask[32:64, :] = labc[None, :]
        in_maps.append({
            "asT": asT_c,
            "embT": emb16,
            "embloc": np.ascontiguousarray(emb16[rows]),
            "idxs": np.ascontiguousarray(
                np.stack([idxtile(src[es]), idxtile(dst[es])], axis=1)),
            "mask": mask,
            "labels_t": np.ascontiguousarray(labc.reshape(NCHUNK, 512)),
            "w2t": w2t, "bstack": bstack, "spm": spm, "oh": oh,
            "ones8": ones8, "ident": ident,
        })
    return in_maps


def kernel(edges, labels, A_s, embedding, W, b, trace=False):
    from concourse.bass_utils import run_bass_kernel_spmd

    in_maps = _host_inputs(edges, labels, A_s, embedding, W, b)
    nc = _get_program()
    res = run_bass_kernel_spmd(nc, in_maps, list(range(NCORES)), trace=trace)
    preds = np.concatenate(
        [res.results[c]["predicts"].reshape(-1) for c in range(NCORES)])
    loss = np.float32(
        sum(float(res.results[c]["loss"][0, 0]) for c in range(NCORES)))
    if trace:
        kernel.last_results = res
    return loss, preds.astype(np.float32)
